# revision 10
# baseline (speedup 1.0000x reference)
"""Multi-head attention Trainium2 Bass kernel.

Problem: B=1, S=4096, D=1024, H=16, Dh=64, f32 I/O.

Sharding: tensor-parallel over heads -- each of the 8 cores computes 2 heads'
q/k/v projections and attention, then an AllToAll redistributes the per-head
outputs so each core holds all 16 heads for a 512-row sequence slice and
computes that slice of the output projection.  No all-reduce needed.

Per-core layout trick: everything is computed "transposed" (head-dim on
partitions) so the kernel needs zero on-chip transposes of activations:
  x^T  [D, S]      (via DMA xbar transpose of the bf16-cast input)
  Q^T,K^T [2*64,S] = w.T @ x^T      (lhsT = w, rhs = x^T)
  S^T  [Sk, Sq]    = K^T.T @ Q^T    (lhsT = K^T tile, rhs = Q^T, K=64,
                                     both heads packed via PE row-tiling)
  P^T  = exp(S^T/8)  on ACT, PSUM->SBUF bf16
  O^T  [65, Sq]    = [V|1].T @ P^T  (lhsT = V with a ones column -> row 64
                                     accumulates the softmax denominator)
  out  [Sq, D]     = sum_r block_r.T @ wo_r   (lhsT = A2A'd O^T blocks)
"""

import sys

if "/opt/trn_rl_repo" not in sys.path:
    sys.path.insert(0, "/opt/trn_rl_repo")

from contextlib import ExitStack

import numpy as np
import ml_dtypes

import concourse.bass as bass
import concourse.bacc as bacc
import concourse.tile as tile
from concourse import mybir
from concourse.bass_utils import run_bass_kernel_spmd

BF16_NP = ml_dtypes.bfloat16
FP32 = mybir.dt.float32
BF16 = mybir.dt.bfloat16
EXP = mybir.ActivationFunctionType.Exp

P = 128          # partitions
S = 4096         # sequence
D = 1024         # model dim
DH = 64          # head dim
NH = 2           # heads per core
NC = 8           # cores
SL = S // NC     # output seq slice per core (512)
QSPAN = 1024     # q-columns processed per attention outer iteration
NKT = S // P     # 32 k-tiles
ND8 = D // P     # 8 contraction chunks


def _bcast_ap(ap, nparts):
    """Broadcast a [N]-shaped DRAM AP across nparts partitions."""
    return bass.AP(
        tensor=ap.tensor, offset=ap.offset, ap=[[0, nparts]] + [list(a) for a in ap.ap]
    )


def build_mha(nc: bass.Bass):
    x_d = nc.dram_tensor("x", [S, D], BF16, kind="ExternalInput")
    wq_d = nc.dram_tensor("wq", [D, NH * DH], BF16, kind="ExternalInput")
    wk_d = nc.dram_tensor("wk", [D, NH * DH], BF16, kind="ExternalInput")
    wv_d = nc.dram_tensor("wv", [D, NH * DH], BF16, kind="ExternalInput")
    bq_d = nc.dram_tensor("bq", [NH * DH], FP32, kind="ExternalInput")
    bk_d = nc.dram_tensor("bk", [NH * DH], FP32, kind="ExternalInput")
    bv_d = nc.dram_tensor("bv", [NH * DH], FP32, kind="ExternalInput")
    wo_d = nc.dram_tensor("wo", [D, D], BF16, kind="ExternalInput")
    bo_d = nc.dram_tensor("bo", [D], FP32, kind="ExternalInput")
    out_d = nc.dram_tensor("out", [SL, D], FP32, kind="ExternalOutput")

    with tile.TileContext(nc) as tc, ExitStack() as ctx:
        persist = ctx.enter_context(tc.tile_pool(name="persist", bufs=1))
        dram = ctx.enter_context(tc.tile_pool(name="dram", bufs=1, space="DRAM"))

        # one tile per d8 chunk, single transpose-DMA writer each (the xpose
        # instruction has very few sync-wait slots)
        xTs = [
            persist.tile([P, S], BF16, tag=f"xT{d8}", name=f"xT{d8}")
            for d8 in range(ND8)
        ]  # xTs[d8][p, s] = x[s, d8*128+p]
        wq_sb = persist.tile([P, ND8, NH * DH], BF16)
        wk_sb = persist.tile([P, ND8, NH * DH], BF16)
        wv_sb = persist.tile([P, ND8, NH * DH], BF16)
        bq_sb = persist.tile([P, 1], FP32)
        bk_sb = persist.tile([P, 1], FP32)
        bv_sb = persist.tile([DH, NH], FP32)       # [dh, h] (both heads on parts 0-63)
        qT = persist.tile([P, S], BF16)            # rows 0-63 head0, 64-127 head1
        kT = persist.tile([P, S], BF16)
        v_sb = persist.tile([P, NKT, 2 * (DH + 1)], BF16)  # [.., h*(64+ones)]
        o_sb = [
            persist.tile([DH, S], BF16, tag=f"osb{h}", name=f"osb{h}")
            for h in range(NH)
        ]
        wo_sb = persist.tile([P, NC, D], BF16)     # [p, r, n] = wo[r*128+p, n]
        bo_sb = persist.tile([P, D], FP32)
        cc_sb = persist.tile([P, NC, SL], BF16)    # A2A result [p, r, s]

        a2a_in = dram.tile([NC, P, SL], BF16)
        a2a_out = dram.tile([NC, P, SL], BF16)
        r_dram = dram.tile([NH, S // QSPAN, QSPAN], FP32)

        # ---------- x^T via DMA xbar transpose ----------
        # These must be the first DMAs issued: the xpose descriptor has a
        # single sync-wait slot, and Tile serializes DMATranspose against any
        # earlier plain DMA on the same HW queue (xbar-mode transition), which
        # overflows that slot.
        for d8 in range(ND8):
            nc.sync.dma_start_transpose(
                out=xTs[d8][:, :], in_=x_d[:, d8 * P : (d8 + 1) * P]
            )

        # ---------- constant loads ----------
        for w_d, w_sb in ((wq_d, wq_sb), (wk_d, wk_sb), (wv_d, wv_sb)):
            nc.gpsimd.dma_start(out=w_sb, in_=w_d[:, :].rearrange("(d8 p) m -> p d8 m", p=P))
        nc.gpsimd.dma_start(out=bq_sb, in_=bq_d[:].rearrange("(p one) -> p one", one=1))
        nc.gpsimd.dma_start(out=bk_sb, in_=bk_d[:].rearrange("(p one) -> p one", one=1))
        nc.gpsimd.dma_start(out=bv_sb, in_=bv_d[:].rearrange("(h p) -> p h", p=DH))
        nc.gpsimd.dma_start(out=wo_sb, in_=wo_d[:, :].rearrange("(r p) n -> p r n", p=P))
        nc.gpsimd.dma_start(out=bo_sb, in_=_bcast_ap(bo_d[:], P))
        # ones columns for the softmax-denominator rows of V'
        nc.vector.memset(v_sb[:, :, DH : DH + 1], 1.0)
        nc.vector.memset(v_sb[:, :, 2 * DH + 1 : 2 * DH + 2], 1.0)

        # ---------- QKV projections ----------
        with tc.tile_pool(name="psqkv", bufs=3, space="PSUM") as psA:
            for w_sb, b_sb, dstT in ((wq_sb, bq_sb, qT), (wk_sb, bk_sb, kT)):
                for qc in range(S // 512):
                    ps = psA.tile([P, 512], FP32, tag="qk")
                    for d8 in range(ND8):
                        nc.tensor.matmul(
                            ps,
                            lhsT=w_sb[:, d8, :],
                            rhs=xTs[d8][:, qc * 512 : (qc + 1) * 512],
                            start=(d8 == 0),
                            stop=(d8 == ND8 - 1),
                        )
                    nc.vector.tensor_scalar_add(
                        out=dstT[:, qc * 512 : (qc + 1) * 512], in0=ps, scalar1=b_sb
                    )
            for kt in range(NKT):
                psv = psA.tile([P, NH * DH], FP32, tag="v")
                for d8 in range(ND8):
                    nc.tensor.matmul(
                        psv,
                        lhsT=xTs[d8][:, kt * P : (kt + 1) * P],
                        rhs=wv_sb[:, d8, :],
                        start=(d8 == 0),
                        stop=(d8 == ND8 - 1),
                    )
                vdst = v_sb[:, kt, :].rearrange("p (g c) -> p g c", c=DH + 1)[:, :, 0:DH]
                nc.vector.tensor_copy(
                    out=vdst, in_=psv[:, :].rearrange("p (g c) -> p g c", c=DH)
                )

        # ---------- attention ----------
        with tc.tile_pool(name="pss", bufs=1, space="PSUM") as psS, \
             tc.tile_pool(name="pso", bufs=1, space="PSUM") as psO, \
             tc.tile_pool(name="pp", bufs=3) as pP, \
             tc.tile_pool(name="pn", bufs=2) as pN:
            for qi in range(S // QSPAN):
                q0 = qi * QSPAN
                po = [
                    psO.tile([DH + 1, QSPAN], FP32, tag=f"o{h}", name=f"po{h}")
                    for h in range(NH)
                ]
                for kt in range(NKT):
                    for h in range(NH):
                        hp = slice(h * DH, (h + 1) * DH)
                        ps = psS.tile([P, QSPAN], FP32, tag=f"s{h}")
                        for hf in range(QSPAN // 512):
                            nc.tensor.matmul(
                                ps[:, hf * 512 : (hf + 1) * 512],
                                lhsT=kT[hp, kt * P : (kt + 1) * P],
                                rhs=qT[hp, q0 + hf * 512 : q0 + (hf + 1) * 512],
                                start=True,
                                stop=True,
                                tile_position=(h * DH, 0),
                            )
                        pt = pP.tile([P, QSPAN], BF16, tag=f"p{h}")
                        nc.scalar.activation(out=pt, in_=ps, func=EXP, scale=0.125)
                        vl = v_sb[:, kt, h * (DH + 1) : (h + 1) * (DH + 1)]
                        for hf in range(QSPAN // 512):
                            nc.tensor.matmul(
                                po[h][:, hf * 512 : (hf + 1) * 512],
                                lhsT=vl,
                                rhs=pt[:, hf * 512 : (hf + 1) * 512],
                                start=(kt == 0),
                                stop=(kt == NKT - 1),
                            )
                # normalize: rows 0-63 are sum(P*V), row 64 is sum(P) = denominator
                for h in range(NH):
                    rc = pN.tile([DH + 1, QSPAN], FP32, tag="rc")
                    nc.vector.reciprocal(out=rc[DH : DH + 1, :], in_=po[h][DH : DH + 1, :])
                    nc.gpsimd.dma_start(
                        out=r_dram[h, qi, :].rearrange("(one s) -> one s", one=1),
                        in_=rc[DH : DH + 1, :],
                    )
                    rb = pN.tile([DH, QSPAN], FP32, tag="rb")
                    nc.gpsimd.dma_start(out=rb, in_=_bcast_ap(r_dram[h, qi, :], DH))
                    ot = pN.tile([DH, QSPAN], FP32, tag="ot")
                    nc.vector.tensor_mul(out=ot, in0=po[h][0:DH, :], in1=rb)
                    nc.vector.tensor_scalar_add(
                        out=o_sb[h][:, q0 : q0 + QSPAN],
                        in0=ot,
                        scalar1=bv_sb[:, h : h + 1],
                    )

        # ---------- AllToAll: heads -> sequence slices ----------
        for h in range(NH):
            for j in range(NC):
                nc.gpsimd.dma_start(
                    out=a2a_in[j, h * DH : (h + 1) * DH, :],
                    in_=o_sb[h][:, j * SL : (j + 1) * SL],
                )
        nc.gpsimd.collective_compute(
            "AllToAll",
            mybir.AluOpType.bypass,
            replica_groups=[list(range(NC))],
            ins=[a2a_in.opt()],
            outs=[a2a_out.opt()],
        )
        for r in range(NC):
            nc.gpsimd.dma_start(out=cc_sb[:, r, :], in_=a2a_out[r, :, :])

        # ---------- output projection for my 512-row slice ----------
        with tc.tile_pool(name="psf", bufs=2, space="PSUM") as psF, \
             tc.tile_pool(name="pf", bufs=2) as pF:
            for st in range(SL // P):
                for nh in range(D // 512):
                    pso = psF.tile([P, 512], FP32, tag="po")
                    for r in range(NC):
                        nc.tensor.matmul(
                            pso,
                            lhsT=cc_sb[:, r, st * P : (st + 1) * P],
                            rhs=wo_sb[:, r, nh * 512 : (nh + 1) * 512],
                            start=(r == 0),
                            stop=(r == NC - 1),
                        )
                    ft = pF.tile([P, 512], FP32, tag="ft")
                    nc.vector.tensor_add(
                        out=ft, in0=pso, in1=bo_sb[:, nh * 512 : (nh + 1) * 512]
                    )
                    nc.gpsimd.dma_start(
                        out=out_d[st * P : (st + 1) * P, nh * 512 : (nh + 1) * 512],
                        in_=ft,
                    )
    return nc


def _make_in_maps(x, wq, bq, wk, bk, wv, bv, wo, bo):
    x_bf = np.ascontiguousarray(x.reshape(S, D)).astype(BF16_NP)
    wo_bf = np.ascontiguousarray(wo).astype(BF16_NP)
    bo_f = np.ascontiguousarray(bo).astype(np.float32)
    in_maps = []
    for c in range(NC):
        hs = [NH * c + i for i in range(NH)]
        im = {
            "x": x_bf,
            "wq": np.concatenate([wq[h] for h in hs], axis=1).astype(BF16_NP),
            "wk": np.concatenate([wk[h] for h in hs], axis=1).astype(BF16_NP),
            "wv": np.concatenate([wv[h] for h in hs], axis=1).astype(BF16_NP),
            "bq": np.concatenate([bq[h] for h in hs]).astype(np.float32),
            "bk": np.concatenate([bk[h] for h in hs]).astype(np.float32),
            "bv": np.concatenate([bv[h] for h in hs]).astype(np.float32),
            "wo": wo_bf,
            "bo": bo_f,
        }
        in_maps.append(im)
    return in_maps


def run(inputs, trace=False, trace_kwargs=None):
    nc = bacc.Bacc(num_devices=NC)
    build_mha(nc)
    nc.finalize()  # Bacc: runs the bacc pipeline (wait splitting, reg alloc)
    in_maps = _make_in_maps(**{k: np.asarray(v) for k, v in inputs.items()})
    res = run_bass_kernel_spmd(
        nc,
        in_maps,
        core_ids=list(range(NC)),
        trace=trace,
        **(trace_kwargs or {}),
    )
    out = np.concatenate([res.results[c]["out"] for c in range(NC)], axis=0)
    return out.reshape(1, S, D).astype(np.float32), res


def kernel(**inputs):
    out, _ = run(inputs, trace=False)
    return out
